# revision 41
# baseline (speedup 1.0000x reference)
"""Multi-head attention (B=4, H=16, S=1024, D=64) on 8 Trainium2 cores.

Sharding: core c -> batch b = c//2, head-half hh = c%2 (8 heads each).

Per-core dataflow (engine-balanced):
  Pool ring : casting SWDGE loads  q,k (f32->bf16), mask (i32->bf16),
              v (f32->bf16, 65th column = ones for row sums)
  PE        : Q/K/mask 128x128 transposes (bf16, via identity),
              scores S_T[k,q] = K.Q^T per (head, kt), ctx_T[d|sum, q]
  ACT       : exp only - 64 calls of [128,1024] PSUM->SBUF bf16
  DVE       : transpose-batch copies PSUM->SBUF, mask multiply in
              [128, 2x1024] chunks, ctx_T copy, recip + broadcasted
              normalize epilogue
  SP ring   : f32 output stores

PSUM: scores 2x2 banks (double buffer) + ctx 2 banks + small 2x1 banks.
"""

import numpy as np

import concourse.bass as bass
import concourse.mybir as mybir
import concourse.tile as tile
from concourse import bacc
from concourse.bass import broadcast_tensor_aps
from concourse.bass_utils import run_bass_kernel_spmd
from concourse.masks import make_identity

F32 = mybir.dt.float32
BF16 = mybir.dt.bfloat16
I32 = mybir.dt.int32

S = 1024          # sequence length
DH = 64           # head dim
HEADS = 8         # heads per core
DCORE = HEADS * DH  # 512, model-dim slice per core
NQT = S // 128    # 8 q blocks
NKT = S // 128    # 8 k blocks
SCALE = 1.0 / 8.0  # 1/sqrt(64)


def build_nc(reps: int = 1, loop: int = 0):
    nc = bacc.Bacc(
        "TRN2", target_bir_lowering=False, debug=False, num_devices=8
    )
    q_in = nc.declare_dram_parameter("q", [S, DCORE], F32, isOutput=False)
    k_in = nc.declare_dram_parameter("k", [S, DCORE], F32, isOutput=False)
    v_in = nc.declare_dram_parameter("v", [S, DCORE], F32, isOutput=False)
    m_in = nc.declare_dram_parameter("mask", [S, S], I32, isOutput=False)
    out = nc.declare_dram_parameter("out", [S, DCORE], F32, isOutput=True)

    with tile.TileContext(nc) as tc:
        with (
            tc.tile_pool(name="persist", bufs=1) as persist,
            tc.tile_pool(name="epool", bufs=8) as epool,
            tc.tile_pool(name="cpool", bufs=2) as cpool,
            tc.tile_pool(name="opool", bufs=2) as opool,
            tc.tile_pool(name="ps_s", bufs=3, space="PSUM") as ps_s_pool,
            tc.tile_pool(name="ps_ctx", bufs=2, space="PSUM") as ps_ctx_pool,
        ):
            # identity is constant: build it once, outside the rep bodies,
            # so reps never WAR-serialize on it
            ident = persist.tile([128, 128], F32, tag="ident", name="ident")
            make_identity(nc, ident[:])
            ident_b16 = persist.tile(
                [128, 128], BF16, tag="identb16", name="identb16"
            )
            nc.vector.tensor_copy(ident_b16[:], ident[:])

            def body(ridx, carry):
                emit_body(
                    nc, tc, q_in, k_in, v_in, m_in, out, ident_b16,
                    persist, epool, cpool, opool,
                    ps_s_pool, ps_ctx_pool, ridx % 2, carry,
                )

            def flush(carry):
                if carry.get("flush_fn") is not None:
                    carry["flush_fn"]()
                    carry["flush_fn"] = None

            if loop:
                with tc.For_i(0, loop, 1):
                    carry = {}
                    for r in range(reps):
                        body(r, carry)
                    flush(carry)  # inside the loop: nothing crosses the backedge
            else:
                carry = {}
                for r in range(reps):
                    body(r, carry)
                flush(carry)
    nc.compile()
    return nc


def emit_body(
    nc, tc, q_in, k_in, v_in, m_in, out, ident_b16,
    persist, epool, cpool, opool, ps_s_pool, ps_ctx_pool, pp, carry,
):
    # ---- staging tiles (bf16, filled by casting SWDGE loads) ----
    q16 = persist.tile([128, NQT, DCORE], BF16, tag="q16", name="q16")
    k16 = persist.tile([128, NKT, DCORE], BF16, tag="k16", name="k16")
    mf = persist.tile([128, NQT, S], BF16, tag="mf", name="mf")
    v_all = persist.tile([128, NKT, HEADS, 65], BF16, tag=f"v{pp}", name=f"v_all{pp}")

    # transposed layouts
    QT = persist.tile([128, 4, S], BF16, tag=f"QT{pp}", name=f"QT{pp}")  # [d, dp, q]
    KT = persist.tile([128, 4, S], BF16, tag=f"KT{pp}", name=f"KT{pp}")  # [d, dp, k]
    maskT = persist.tile([128, NKT, S], BF16, tag=f"maskT{pp}", name=f"maskT{pp}")

    # loads: q/k in sb-halves (transposes start on half 0), mask split by
    # k-columns (kt 0-3 usable early), v per head (h0/h1 early for ctx)
    qr = q_in.rearrange("(sb p) d -> p sb d", p=128)
    kr = k_in.rearrange("(sb p) d -> p sb d", p=128)
    mr = m_in.rearrange("(qb p) k -> p qb k", p=128)

    def v_load(h):
        nc.gpsimd.dma_start(
            v_all[:, :, h, 0:64],
            v_in[:, h * 64:(h + 1) * 64].rearrange("(kt p) d -> p kt d", p=128),
        )

    nc.gpsimd.dma_start(q16[:, 0:4, :], qr[:, 0:4, :])
    nc.gpsimd.dma_start(k16[:, 0:4, :], kr[:, 0:4, :])
    nc.gpsimd.dma_start(q16[:, 4:8, :], qr[:, 4:8, :])
    nc.gpsimd.dma_start(k16[:, 4:8, :], kr[:, 4:8, :])
    nc.gpsimd.dma_start(mf[:, :, 0:512], mr[:, :, 0:512])
    nc.gpsimd.dma_start(mf[:, :, 512:1024], mr[:, :, 512:1024])
    for h in range(HEADS):
        v_load(h)
    nc.vector.memset(v_all[:, :, :, 64:65], 1.0)

    # ---- transposes on PE, batch-copied to SBUF by DVE ----
    # dp=0 first (first heads), then all mask kts (head 0 needs every kt),
    # then the later dps
    def qk_transpose(dp):
        for src, dst in ((q16, QT), (k16, KT)):
            if dp == 0:
                ps = ps_s_pool.tile([128, 8, 128], BF16, tag="s", name="ps_qk")
            else:
                ps = ps_ctx_pool.tile([128, 8, 128], BF16, tag="ctx", name="ps_qk")
            for sb in range(8):
                nc.tensor.transpose(
                    ps[:, sb, :], src[:, sb, dp * 128:(dp + 1) * 128],
                    ident_b16[:],
                )
            nc.vector.tensor_copy(dst[:, dp, :], ps[:])

    def mask_transpose(kt):
        ps = ps_ctx_pool.tile([128, 8, 128], BF16, tag="ctx", name="ps_m")
        for qb in range(8):
            nc.tensor.transpose(
                ps[:, qb, :], mf[:, qb, kt * 128:(kt + 1) * 128], ident_b16[:]
            )
        nc.vector.tensor_copy(maskT[:, kt, :], ps[:])

    qk_transpose(0)
    mask_transpose(0)
    mask_transpose(1)
    mask_transpose(2)
    mask_transpose(3)
    qk_transpose(1)
    mask_transpose(4)
    mask_transpose(5)
    mask_transpose(6)
    mask_transpose(7)
    qk_transpose(2)
    qk_transpose(3)

    # ---- per-head attention, software-pipelined emission ----
    # ctx matmuls for chunk (h, kc) are emitted AFTER the next chunk's
    # scores/exp, so the PE priority order always favors the scores that
    # feed ACT (the bottleneck engine); ctx fills PE slack afterwards.
    ctx_state = {"tiles": None, "h": None}

    def ctx_tiles_for(h):
        if ctx_state["h"] != h:
            ctx_state["tiles"] = [
                ps_ctx_pool.tile([65, 512], F32, tag="ctx", name=f"ps_ctx{qc}")
                for qc in range(2)
            ]
            ctx_state["h"] = h
        return ctx_state["tiles"]

    def emit_ctx(h, kc, e2):
        ps_ctx = ctx_tiles_for(h)
        for j in range(2):
            kt = 2 * kc + j
            for qc in range(2):
                nc.tensor.matmul(
                    ps_ctx[qc][:, :],
                    lhsT=v_all[:, kt, h, :],
                    rhs=e2[:, j, qc * 512:(qc + 1) * 512],
                    start=(kt == 0), stop=(kt == NKT - 1),
                )

    def emit_epilogue(h):
        ps_ctx = ctx_state["tiles"]
        ctxT = cpool.tile([65, S], BF16, tag="ctxT", name="ctxT")
        nc.vector.tensor_copy(ctxT[:, 0:512], ps_ctx[0][:, :])
        nc.vector.tensor_copy(ctxT[:, 512:1024], ps_ctx[1][:, :])
        ps_o = ps_ctx_pool.tile([128, 8, 66], BF16, tag="ctx", name="ps_o")
        for qb in range(NQT):
            nc.tensor.transpose(
                ps_o[:, qb, 0:65], ctxT[:, qb * 128:(qb + 1) * 128],
                ident_b16[:65, :65],
            )
        recip8 = opool.tile([128, 8, 1], F32, tag="r", name="recip8")
        nc.vector.reciprocal(recip8[:], ps_o[:, :, 64:65])
        o_head = opool.tile([128, 8, 64], F32, tag="o", name="o_head")
        num_ap, rec_ap = broadcast_tensor_aps(ps_o[:, :, 0:64], recip8[:])
        nc.vector.tensor_mul(o_head[:], num_ap, rec_ap)
        nc.sync.dma_start(
            out[:, h * 64:(h + 1) * 64].rearrange("(qb p) d -> p qb d", p=128),
            o_head[:],
        )

    pending = None  # (h, kc, e2) whose ctx matmuls are not yet emitted
    first_chunk = True
    for h in range(HEADS):
        dp = h // 2
        hp = (h % 2) * 64  # partition base of this head inside the pair tile
        for kc in range(NKT // 2):  # kt pairs
            e2 = epool.tile([128, 2, S], BF16, tag="e", name="e2")
            for j in range(2):
                kt = 2 * kc + j
                ps_s = ps_s_pool.tile([128, S], F32, tag="s", name="ps_s")
                for qc in range(2):
                    nc.tensor.matmul(
                        ps_s[:, qc * 512:(qc + 1) * 512],
                        lhsT=KT[hp:hp + 64, dp, kt * 128:(kt + 1) * 128],
                        rhs=QT[hp:hp + 64, dp, qc * 512:(qc + 1) * 512],
                        start=True,
                        stop=True,
                    )
                nc.scalar.activation(
                    e2[:, j, :],
                    ps_s[:],
                    mybir.ActivationFunctionType.Exp,
                    scale=SCALE,
                )
            # mask both kt chunks in one 2x1024 multiply (bf16 2x mode)
            nc.vector.tensor_mul(
                e2[:, :, :], e2[:, :, :], maskT[:, 2 * kc:2 * kc + 2, :]
            )
            if first_chunk:
                # previous rep's deferred tail (its closures, its tiles)
                if carry.get("flush_fn") is not None:
                    carry["flush_fn"]()
                    carry["flush_fn"] = None
                first_chunk = False
            if pending is not None:
                ph, pkc, pe2 = pending
                emit_ctx(ph, pkc, pe2)
                if pkc == NKT // 2 - 1:
                    emit_epilogue(ph)
            pending = (h, kc, e2)

    def _flush_tail(p=pending):
        ph, pkc, pe2 = p
        emit_ctx(ph, pkc, pe2)
        if pkc == NKT // 2 - 1:
            emit_epilogue(ph)

    carry["flush_fn"] = _flush_tail


_NC_CACHE = None


def kernel(query, key, value, attention_mask):
    global _NC_CACHE
    query = np.asarray(query, dtype=np.float32)
    key = np.asarray(key, dtype=np.float32)
    value = np.asarray(value, dtype=np.float32)
    attention_mask = np.asarray(attention_mask, dtype=np.int32)

    B = query.shape[0]
    in_maps = []
    for c in range(8):
        b, hh = c // 2, c % 2
        sl = slice(hh * DCORE, (hh + 1) * DCORE)
        in_maps.append(
            {
                "q": np.ascontiguousarray(query[b, :, sl]),
                "k": np.ascontiguousarray(key[b, :, sl]),
                "v": np.ascontiguousarray(value[b, :, sl]),
                "mask": np.ascontiguousarray(attention_mask[b]),
            }
        )

    if _NC_CACHE is None:
        _NC_CACHE = build_nc()
    res = run_bass_kernel_spmd(_NC_CACHE, in_maps, core_ids=list(range(8)))

    outf = np.empty((B, S, 2 * DCORE), np.float32)
    for c in range(8):
        b, hh = c // 2, c % 2
        outf[b, :, hh * DCORE:(hh + 1) * DCORE] = res.results[c]["out"]
    return outf


if __name__ == "__main__":
    rng = np.random.default_rng(0)
    q = rng.standard_normal((4, S, 1024), dtype=np.float32)
    k = rng.standard_normal((4, S, 1024), dtype=np.float32)
    v = rng.standard_normal((4, S, 1024), dtype=np.float32)
    m = rng.integers(0, 2, size=(4, S, S)).astype(np.int32)
    o = kernel(q, k, v, m)
    print(o.shape, o.dtype)
